# revision 12
# baseline (speedup 1.0000x reference)
"""Conditional BatchNorm1d (training mode) on 8 Trainium2 NeuronCores.

Strategy (feature-parallel, sort-packed, zero collectives):
  - Host: stable-sort rows by label; split each label's rows evenly over 8
    row-chunks; pack a per-core [128, 16*CAP] bf16 array where partition
    p = chunk*16 + i holds feature (16*core + i) of that chunk's rows,
    one fixed-capacity zero-padded column slot per label.
  - Each core owns 16 of the 128 features for ALL 500k rows, so the global
    per-label statistics are computed entirely locally -- no AllReduce.
  - Pass 1 rides mostly on the otherwise-idle PE: per label slot, matmuls
    with a [128,16] chunk-selector lhsT accumulate 248-column groups into
    PSUM (the free-axis reduction becomes PSUM accumulation), then a short
    DVE tensor_reduce collapses [16,248] -> [16,1]. sum(x^2) needs x^2 in
    SBUF: DVE squares 7 slots (2x-rate scalar_tensor_tensor) for the PE,
    the scalar engine handles 9 slots directly via Square + accum_out.
  - Scale/shift are computed on 16 partitions (mirroring the reference
    formulas) and broadcast to 128 partitions with one tiny fp32 matmul.
  - Pass 2: per label slot, y = x*scale + shift as a single per-partition
    tensor_scalar (DVE) or activation Identity (scalar engine), DMA out.
  - x stays SBUF-resident between the passes: HBM traffic is ~16 MB in +
    ~16 MB out per core (bf16). All bulk DMAs ride the sync/gpsimd rings;
    the scalar engine issues none (ring-FIFO backpressure would stall its
    compute stream). gpsimd runs no compute: it shares an SBUF port with
    DVE (exclusive lock) and would stall it.

Everything hardcoded for: x [500000,128] f32, labels [500000] int,
gamma/beta [16,128] f32, 16 conditions. bf16 data path keeps the norm
relative error ~2e-3, well inside the 2e-2 gate.
"""
import numpy as np

N_CORES = 8
N = 500000
F = 128
C = 16
EPS = 1e-5

FPC = F // N_CORES     # 16 features owned per core
NJ = 8                 # row-chunks -> partitions = NJ*FPC = 128
P = NJ * FPC           # 128
WP = 496               # PSUM accumulation width (cap must be a multiple)
ALIGN = 8 * WP         # keeps cap a multiple of WP (and of 64)

# pass-1 split (early slots arrive first; late slots need low-latency
# single-op paths so the stats tail after the last DMA stays short):
SX_PE = frozenset(range(0, 8))       # sum(x) on PE; the rest fold on DVE
SQ_PE = frozenset({10, 11, 12, 13})  # sum(x^2): DVE-squared, PE-summed
# sum(x^2) for the rest rides the scalar engine: slots 0-9 drain its queue
# early so 14/15 get the low-latency path right after their DMAs land
# pass-2 apply: these label slots go to the scalar engine
AP_SC = frozenset({6, 12})

_CACHE = {}


def _build(cap):
    import concourse.bacc as bacc
    import concourse.bass as bass
    from concourse import mybir
    import concourse.tile as tile

    F32 = mybir.dt.float32
    BF16 = mybir.dt.bfloat16
    AF = mybir.ActivationFunctionType
    ALU = mybir.AluOpType
    cols = C * cap
    G = cap // WP

    nc = bacc.Bacc("TRN2", target_bir_lowering=False, debug=False,
                   num_devices=N_CORES)
    xt = nc.dram_tensor("xt", [P, cols], BF16, kind="ExternalInput").ap()
    gbt = nc.dram_tensor("gbt", [FPC, 3 * C], F32, kind="ExternalInput").ap()
    selb = nc.dram_tensor("selb", [P, FPC], BF16, kind="ExternalInput").ap()
    self32 = nc.dram_tensor("self32", [P, FPC], F32, kind="ExternalInput").ap()
    rep32 = nc.dram_tensor("rep32", [FPC, P], F32, kind="ExternalInput").ap()
    y = nc.dram_tensor("y", [P, cols], BF16, kind="ExternalOutput").ap()

    def x_slice(s):
        return bass.AP(tensor=xt.tensor, offset=s * cap,
                       ap=[[cols, P], [1, cap]])

    def y_slice(s):
        return bass.AP(tensor=y.tensor, offset=s * cap,
                       ap=[[cols, P], [1, cap]])

    with tile.TileContext(nc) as tc:
        with (
            tc.tile_pool(name="const", bufs=1) as const,
            tc.tile_pool(name="xres", bufs=C) as xres,
            tc.tile_pool(name="sqv", bufs=2) as sqv,
            tc.tile_pool(name="sqs", bufs=2) as sqs,
            tc.tile_pool(name="f1p", bufs=2) as f1p,
            tc.tile_pool(name="f2p", bufs=2) as f2p,
            tc.tile_pool(name="ybuf", bufs=4) as ybuf,
            tc.tile_pool(name="psx", bufs=3, space="PSUM") as psx,
            tc.tile_pool(name="psq", bufs=2, space="PSUM") as psq,
            tc.tile_pool(name="psg", bufs=1, space="PSUM") as psg,
        ):
            # ---- constants (small, scalar ring so they land first) ----
            gbt_sb = const.tile([FPC, 3 * C], F32)
            nc.scalar.dma_start(out=gbt_sb[:], in_=gbt)
            selb_sb = const.tile([P, FPC], BF16)
            nc.scalar.dma_start(out=selb_sb[:], in_=selb)
            self_sb = const.tile([P, FPC], F32)
            nc.scalar.dma_start(out=self_sb[:], in_=self32)
            rep_sb = const.tile([FPC, P], F32)
            nc.scalar.dma_start(out=rep_sb[:], in_=rep32)
            eps_sb = const.tile([FPC, 1], F32)
            nc.vector.memset(eps_sb[:], EPS)

            # ---- resident x: one [P, cap] tile per label slot ----
            xts = []
            for s in range(C):
                t = xres.tile([P, cap], BF16, tag="x")
                eng = nc.sync if s % 2 == 0 else nc.gpsimd
                eng.dma_start(out=t[:], in_=x_slice(s))
                xts.append(t)

            # ---- pass 1 ----
            # s16[i, 0:C) = global sum(x), [C:2C) = global sum(x^2), folded
            # over the 8 chunks. PE passes land there directly; DVE/scalar
            # produce per-partition partials in part_u, folded at the end
            # by one tiny matmul.
            s16 = const.tile([FPC, 2 * C], F32)
            part_u = const.tile([P, 2 * C], F32)
            nc.vector.memset(part_u[:], 0.0)
            h2, h4 = cap // 2, cap // 4

            def pe_slot_sum(src_ap, out_col):
                ps = (psx if out_col < C else psq).tile(
                    [FPC, WP], F32, tag="acc" if out_col < C else "accq")
                for g in range(G):
                    nc.tensor.matmul(out=ps[:], lhsT=selb_sb[:],
                                     rhs=src_ap[:, g * WP:(g + 1) * WP],
                                     start=(g == 0), stop=(g == G - 1))
                nc.vector.tensor_reduce(s16[:, out_col:out_col + 1], ps[:],
                                        axis=mybir.AxisListType.X, op=ALU.add)

            for s in range(C):
                if s in SX_PE:
                    pe_slot_sum(xts[s], s)                   # sum(x) on PE
                else:
                    # sum(x) on DVE: two 2x-rate bf16 half-folds + reduce
                    f1 = f1p.tile([P, h2], BF16, tag="f1")
                    nc.vector.tensor_tensor(out=f1[:], in0=xts[s][:, 0:h2],
                                            in1=xts[s][:, h2:cap], op=ALU.add)
                    f2 = f2p.tile([P, h4], BF16, tag="f2")
                    nc.vector.tensor_tensor(out=f2[:], in0=f1[:, 0:h4],
                                            in1=f1[:, h4:h2], op=ALU.add)
                    nc.vector.tensor_reduce(part_u[:, s:s + 1], f2[:],
                                            axis=mybir.AxisListType.X,
                                            op=ALU.add)
                if s in SQ_PE:
                    sq = sqv.tile([P, cap], BF16, tag="scrv")
                    nc.vector.tensor_tensor(out=sq[:], in0=xts[s][:],
                                            in1=xts[s][:], op=ALU.mult)
                    pe_slot_sum(sq, C + s)                   # sum(x^2) on PE
                else:
                    scr = sqs.tile([P, cap], BF16, tag="scrs")
                    nc.scalar.activation(out=scr[:], in_=xts[s][:],
                                         func=AF.Square,
                                         accum_out=part_u[:, C + s:C + s + 1])

            # fold the per-partition partials over the 8 chunks, then copy
            # each contiguous run of non-PE columns into s16
            lo = min(s for s in range(C) if s not in SX_PE)
            psf = psg.tile([FPC, 2 * C - lo], F32, tag="fold")
            nc.tensor.matmul(out=psf[:], lhsT=self_sb[:],
                             rhs=part_u[:, lo:2 * C], start=True, stop=True)
            cols_used = sorted(
                [s for s in range(lo, C) if s not in SX_PE] +
                [C + s for s in range(C) if s not in SQ_PE])
            runs, r0 = [], cols_used[0]
            for a, b in zip(cols_used, cols_used[1:] + [None]):
                if b != a + 1:
                    runs.append((r0, a + 1))
                    r0 = b
            for a, b in runs:
                nc.vector.tensor_copy(out=s16[:, a:b],
                                      in_=psf[:, a - lo:b - lo])

            # ---- stats -> scale/shift on 16 partitions ----
            invn = gbt_sb[:, 2 * C:3 * C]
            meanc = const.tile([FPC, C], F32)
            nc.vector.tensor_tensor(out=meanc[:], in0=s16[:, 0:C],
                                    in1=invn, op=ALU.mult)
            ex2 = const.tile([FPC, C], F32)
            nc.vector.tensor_tensor(out=ex2[:], in0=s16[:, C:2 * C],
                                    in1=invn, op=ALU.mult)
            varc = const.tile([FPC, C], F32)
            nc.vector.tensor_tensor(out=varc[:], in0=meanc[:], in1=meanc[:],
                                    op=ALU.mult)
            nc.vector.tensor_tensor(out=varc[:], in0=ex2[:], in1=varc[:],
                                    op=ALU.subtract)
            stdc = const.tile([FPC, C], F32)
            nc.scalar.activation(out=stdc[:], in_=varc[:], func=AF.Sqrt,
                                 bias=eps_sb[:])
            istd = const.tile([FPC, C], F32)
            nc.vector.reciprocal(out=istd[:], in_=stdc[:])
            scsh = const.tile([FPC, 2 * C], F32)
            nc.vector.tensor_tensor(out=scsh[:, 0:C], in0=gbt_sb[:, 0:C],
                                    in1=istd[:], op=ALU.mult)
            msc = const.tile([FPC, C], F32)
            nc.vector.tensor_tensor(out=msc[:], in0=meanc[:],
                                    in1=scsh[:, 0:C], op=ALU.mult)
            nc.vector.tensor_tensor(out=scsh[:, C:2 * C], in0=gbt_sb[:, C:2 * C],
                                    in1=msc[:], op=ALU.subtract)

            # broadcast scale/shift to all 128 partitions (tiny fp32 matmul)
            psB = psg.tile([P, 2 * C], F32, tag="rep")
            nc.tensor.matmul(out=psB[:], lhsT=rep_sb[:], rhs=scsh[:],
                             start=True, stop=True)
            ss = const.tile([P, 2 * C], F32)
            nc.vector.tensor_copy(out=ss[:], in_=psB[:])

            # ---- pass 2: y = x*scale + shift, slot by slot ----
            for s in range(C):
                yb = ybuf.tile([P, cap], BF16, tag="y")
                if s in AP_SC:
                    nc.scalar.activation(out=yb[:], in_=xts[s][:],
                                         func=AF.Identity,
                                         bias=ss[:, C + s:C + s + 1],
                                         scale=ss[:, s:s + 1])
                else:
                    nc.vector.tensor_scalar(out=yb[:], in0=xts[s][:],
                                            scalar1=ss[:, s:s + 1],
                                            scalar2=ss[:, C + s:C + s + 1],
                                            op0=ALU.mult, op1=ALU.add)
                eng = nc.sync if s % 2 == 0 else nc.gpsimd
                eng.dma_start(out=y_slice(s), in_=yb[:])
    nc.finalize()
    return nc


def _get_nc(cap):
    key = ("nc", cap)
    if key not in _CACHE:
        _CACHE[key] = _build(cap)
    return _CACHE[key]


def kernel(x, labels, gamma, beta):
    import ml_dtypes
    from concourse.bass_utils import run_bass_kernel_spmd

    BF = ml_dtypes.bfloat16
    x = np.asarray(x, dtype=np.float32)
    lab = np.asarray(labels).astype(np.int64).ravel()
    gamma = np.asarray(gamma, dtype=np.float32)
    beta = np.asarray(beta, dtype=np.float32)

    counts = np.bincount(lab, minlength=C).astype(np.int64)
    base, rem = counts // NJ, counts % NJ
    ncj = base[None, :] + (np.arange(NJ)[:, None] < rem[None, :])  # [NJ, C]
    cap = int(-(-int(ncj.max()) // ALIGN) * ALIGN)
    cols = C * cap

    order = np.argsort(lab, kind="stable")
    starts = np.zeros(C + 1, np.int64)
    starts[1:] = np.cumsum(counts)
    # col_idx[j, c*cap + t] = original row index (N -> zero/garbage row)
    col_idx = np.full((NJ, cols), N, dtype=np.int64)
    for c in range(C):
        off = starts[c]
        for j in range(NJ):
            m = int(ncj[j, c])
            col_idx[j, c * cap:c * cap + m] = order[off:off + m]
            off += m

    xb = np.concatenate([x.astype(BF), np.zeros((1, F), BF)], axis=0)
    g = xb[col_idx.reshape(-1)]                    # [NJ*cols, F] bf16
    g = g.reshape(NJ, cols, F).transpose(0, 2, 1)  # [NJ, F, cols]

    invn = (1.0 / np.maximum(counts, 1)).astype(np.float32)
    gT, bT = gamma.T, beta.T                       # [F, C]
    selm = (np.arange(P)[:, None] % FPC == np.arange(FPC)[None, :])
    selm = np.ascontiguousarray(selm.astype(np.float32))

    nc = _get_nc(cap)
    in_maps = []
    for k in range(N_CORES):
        f0 = k * FPC
        xk = np.ascontiguousarray(g[:, f0:f0 + FPC, :]).reshape(P, cols)
        gbk = np.concatenate(
            [gT[f0:f0 + FPC], bT[f0:f0 + FPC],
             np.broadcast_to(invn[None, :], (FPC, C))], axis=1,
        ).astype(np.float32)
        in_maps.append({
            "xt": xk,
            "gbt": np.ascontiguousarray(gbk),
            "selb": selm.astype(BF),
            "self32": selm,
            "rep32": np.ascontiguousarray(selm.T),
        })
    res = run_bass_kernel_spmd(nc, in_maps, core_ids=list(range(N_CORES)),
                               **_CACHE.get("run_kwargs", {}))
    _CACHE["last_results"] = res

    ys = np.empty((N + 1, F), dtype=np.float32)    # row N absorbs padding
    for k in range(N_CORES):
        f0 = k * FPC
        yk = np.asarray(res.results[k]["y"]).reshape(NJ, FPC, cols)
        yk = yk.transpose(0, 2, 1).astype(np.float32)  # [NJ, cols, FPC]
        for j in range(NJ):
            ys[col_idx[j], f0:f0 + FPC] = yk[j]
    return np.ascontiguousarray(ys[:N])


# revision 13
# speedup vs baseline: 1.0964x; 1.0964x over previous
"""Conditional BatchNorm1d (training mode) on 8 Trainium2 NeuronCores.

Strategy (feature-parallel, sort-packed, zero collectives):
  - Host: stable-sort rows by label; split each label's rows evenly over 8
    row-chunks; pack a per-core [128, 16*CAP] bf16 array where partition
    p = chunk*16 + i holds feature (16*core + i) of that chunk's rows,
    one fixed-capacity zero-padded column slot per label.
  - Each core owns 16 of the 128 features for ALL 500k rows, so the global
    per-label statistics are computed entirely locally -- no AllReduce.
  - Pass 1 rides mostly on the otherwise-idle PE: per label slot, matmuls
    with a [128,16] chunk-selector lhsT accumulate 248-column groups into
    PSUM (the free-axis reduction becomes PSUM accumulation), then a short
    DVE tensor_reduce collapses [16,248] -> [16,1]. sum(x^2) needs x^2 in
    SBUF: DVE squares 7 slots (2x-rate scalar_tensor_tensor) for the PE,
    the scalar engine handles 9 slots directly via Square + accum_out.
  - Scale/shift are computed on 16 partitions (mirroring the reference
    formulas) and broadcast to 128 partitions with one tiny fp32 matmul.
  - Pass 2: per label slot, y = x*scale + shift as a single per-partition
    tensor_scalar (DVE) or activation Identity (scalar engine), DMA out.
  - x stays SBUF-resident between the passes: HBM traffic is ~16 MB in +
    ~16 MB out per core (bf16). All bulk DMAs ride the sync/gpsimd rings;
    the scalar engine issues none (ring-FIFO backpressure would stall its
    compute stream). gpsimd runs no compute: it shares an SBUF port with
    DVE (exclusive lock) and would stall it.

Everything hardcoded for: x [500000,128] f32, labels [500000] int,
gamma/beta [16,128] f32, 16 conditions. bf16 data path keeps the norm
relative error ~2e-3, well inside the 2e-2 gate.
"""
import numpy as np

N_CORES = 8
N = 500000
F = 128
C = 16
EPS = 1e-5

FPC = F // N_CORES     # 16 features owned per core
NJ = 8                 # row-chunks -> partitions = NJ*FPC = 128
P = NJ * FPC           # 128
WP = 496               # PSUM accumulation width (cap must be a multiple)
ALIGN = 8 * WP         # keeps cap a multiple of WP (and of 64)

# pass-1 split (early slots arrive first; late slots need low-latency
# single-op paths so the stats tail after the last DMA stays short):
SX_PE = frozenset(range(C))          # sum(x): all slots on the PE
SQ_PE = frozenset({12, 13, 14, 15})  # sum(x^2): DVE-squared, PE-summed
# sum(x^2) for slots 0-11 rides the scalar engine (Square + accum_out)
# pass-2 apply: these label slots go to the scalar engine
AP_SC = frozenset({6, 12})

_CACHE = {}


def _build(cap):
    import concourse.bacc as bacc
    import concourse.bass as bass
    from concourse import mybir
    import concourse.tile as tile

    F32 = mybir.dt.float32
    BF16 = mybir.dt.bfloat16
    AF = mybir.ActivationFunctionType
    ALU = mybir.AluOpType
    cols = C * cap
    G = cap // WP

    nc = bacc.Bacc("TRN2", target_bir_lowering=False, debug=False,
                   num_devices=N_CORES)
    xt = nc.dram_tensor("xt", [P, cols], BF16, kind="ExternalInput").ap()
    gbt = nc.dram_tensor("gbt", [FPC, 3 * C], F32, kind="ExternalInput").ap()
    selb = nc.dram_tensor("selb", [P, FPC], BF16, kind="ExternalInput").ap()
    self32 = nc.dram_tensor("self32", [P, FPC], F32, kind="ExternalInput").ap()
    rep32 = nc.dram_tensor("rep32", [FPC, P], F32, kind="ExternalInput").ap()
    y = nc.dram_tensor("y", [P, cols], BF16, kind="ExternalOutput").ap()

    def x_slice(s):
        return bass.AP(tensor=xt.tensor, offset=s * cap,
                       ap=[[cols, P], [1, cap]])

    def y_slice(s):
        return bass.AP(tensor=y.tensor, offset=s * cap,
                       ap=[[cols, P], [1, cap]])

    with tile.TileContext(nc) as tc:
        with (
            tc.tile_pool(name="const", bufs=1) as const,
            tc.tile_pool(name="xres", bufs=C) as xres,
            tc.tile_pool(name="sqv", bufs=2) as sqv,
            tc.tile_pool(name="sqs", bufs=2) as sqs,
            tc.tile_pool(name="f1p", bufs=2) as f1p,
            tc.tile_pool(name="f2p", bufs=2) as f2p,
            tc.tile_pool(name="ybuf", bufs=4) as ybuf,
            tc.tile_pool(name="psx", bufs=3, space="PSUM") as psx,
            tc.tile_pool(name="psq", bufs=2, space="PSUM") as psq,
            tc.tile_pool(name="psg", bufs=1, space="PSUM") as psg,
        ):
            # ---- constants (small, scalar ring so they land first) ----
            gbt_sb = const.tile([FPC, 3 * C], F32)
            nc.scalar.dma_start(out=gbt_sb[:], in_=gbt)
            selb_sb = const.tile([P, FPC], BF16)
            nc.scalar.dma_start(out=selb_sb[:], in_=selb)
            self_sb = const.tile([P, FPC], F32)
            nc.scalar.dma_start(out=self_sb[:], in_=self32)
            rep_sb = const.tile([FPC, P], F32)
            nc.scalar.dma_start(out=rep_sb[:], in_=rep32)
            eps_sb = const.tile([FPC, 1], F32)
            nc.vector.memset(eps_sb[:], EPS)

            # ---- resident x: one [P, cap] tile per label slot ----
            xts = []
            for s in range(C):
                t = xres.tile([P, cap], BF16, tag="x")
                eng = nc.sync if s % 2 == 0 else nc.gpsimd
                eng.dma_start(out=t[:], in_=x_slice(s))
                xts.append(t)

            # ---- pass 1 ----
            # s16[i, 0:C) = global sum(x), [C:2C) = global sum(x^2), folded
            # over the 8 chunks. PE passes land there directly; DVE/scalar
            # produce per-partition partials in part_u, folded at the end
            # by one tiny matmul.
            s16 = const.tile([FPC, 2 * C], F32)
            part_u = const.tile([P, 2 * C], F32)
            nc.vector.memset(part_u[:], 0.0)
            h2, h4 = cap // 2, cap // 4

            def pe_slot_sum(src_ap, out_col):
                ps = (psx if out_col < C else psq).tile(
                    [FPC, WP], F32, tag="acc" if out_col < C else "accq")
                for g in range(G):
                    nc.tensor.matmul(out=ps[:], lhsT=selb_sb[:],
                                     rhs=src_ap[:, g * WP:(g + 1) * WP],
                                     start=(g == 0), stop=(g == G - 1))
                nc.vector.tensor_reduce(s16[:, out_col:out_col + 1], ps[:],
                                        axis=mybir.AxisListType.X, op=ALU.add)

            for s in range(C):
                if s in SX_PE:
                    pe_slot_sum(xts[s], s)                   # sum(x) on PE
                else:
                    # sum(x) on DVE: two 2x-rate bf16 half-folds + reduce
                    f1 = f1p.tile([P, h2], BF16, tag="f1")
                    nc.vector.tensor_tensor(out=f1[:], in0=xts[s][:, 0:h2],
                                            in1=xts[s][:, h2:cap], op=ALU.add)
                    f2 = f2p.tile([P, h4], BF16, tag="f2")
                    nc.vector.tensor_tensor(out=f2[:], in0=f1[:, 0:h4],
                                            in1=f1[:, h4:h2], op=ALU.add)
                    nc.vector.tensor_reduce(part_u[:, s:s + 1], f2[:],
                                            axis=mybir.AxisListType.X,
                                            op=ALU.add)
                if s in SQ_PE:
                    sq = sqv.tile([P, cap], BF16, tag="scrv")
                    nc.vector.tensor_tensor(out=sq[:], in0=xts[s][:],
                                            in1=xts[s][:], op=ALU.mult)
                    pe_slot_sum(sq, C + s)                   # sum(x^2) on PE
                else:
                    scr = sqs.tile([P, cap], BF16, tag="scrs")
                    nc.scalar.activation(out=scr[:], in_=xts[s][:],
                                         func=AF.Square,
                                         accum_out=part_u[:, C + s:C + s + 1])

            # fold the per-partition partials over the 8 chunks, then copy
            # each contiguous run of non-PE columns into s16
            lo = min([s for s in range(C) if s not in SX_PE] +
                     [C + s for s in range(C) if s not in SQ_PE])
            psf = psg.tile([FPC, 2 * C - lo], F32, tag="fold")
            nc.tensor.matmul(out=psf[:], lhsT=self_sb[:],
                             rhs=part_u[:, lo:2 * C], start=True, stop=True)
            cols_used = sorted(
                [s for s in range(lo, C) if s not in SX_PE] +
                [C + s for s in range(C) if s not in SQ_PE])
            runs, r0 = [], cols_used[0]
            for a, b in zip(cols_used, cols_used[1:] + [None]):
                if b != a + 1:
                    runs.append((r0, a + 1))
                    r0 = b
            for a, b in runs:
                nc.vector.tensor_copy(out=s16[:, a:b],
                                      in_=psf[:, a - lo:b - lo])

            # ---- stats -> scale/shift on 16 partitions ----
            invn = gbt_sb[:, 2 * C:3 * C]
            meanc = const.tile([FPC, C], F32)
            nc.vector.tensor_tensor(out=meanc[:], in0=s16[:, 0:C],
                                    in1=invn, op=ALU.mult)
            ex2 = const.tile([FPC, C], F32)
            nc.vector.tensor_tensor(out=ex2[:], in0=s16[:, C:2 * C],
                                    in1=invn, op=ALU.mult)
            varc = const.tile([FPC, C], F32)
            nc.vector.tensor_tensor(out=varc[:], in0=meanc[:], in1=meanc[:],
                                    op=ALU.mult)
            nc.vector.tensor_tensor(out=varc[:], in0=ex2[:], in1=varc[:],
                                    op=ALU.subtract)
            stdc = const.tile([FPC, C], F32)
            nc.scalar.activation(out=stdc[:], in_=varc[:], func=AF.Sqrt,
                                 bias=eps_sb[:])
            istd = const.tile([FPC, C], F32)
            nc.vector.reciprocal(out=istd[:], in_=stdc[:])
            scsh = const.tile([FPC, 2 * C], F32)
            nc.vector.tensor_tensor(out=scsh[:, 0:C], in0=gbt_sb[:, 0:C],
                                    in1=istd[:], op=ALU.mult)
            msc = const.tile([FPC, C], F32)
            nc.vector.tensor_tensor(out=msc[:], in0=meanc[:],
                                    in1=scsh[:, 0:C], op=ALU.mult)
            nc.vector.tensor_tensor(out=scsh[:, C:2 * C], in0=gbt_sb[:, C:2 * C],
                                    in1=msc[:], op=ALU.subtract)

            # broadcast scale/shift to all 128 partitions (tiny fp32 matmul)
            psB = psg.tile([P, 2 * C], F32, tag="rep")
            nc.tensor.matmul(out=psB[:], lhsT=rep_sb[:], rhs=scsh[:],
                             start=True, stop=True)
            ss = const.tile([P, 2 * C], F32)
            nc.vector.tensor_copy(out=ss[:], in_=psB[:])

            # ---- pass 2: y = x*scale + shift, slot by slot ----
            for s in range(C):
                yb = ybuf.tile([P, cap], BF16, tag="y")
                if s in AP_SC:
                    nc.scalar.activation(out=yb[:], in_=xts[s][:],
                                         func=AF.Identity,
                                         bias=ss[:, C + s:C + s + 1],
                                         scale=ss[:, s:s + 1])
                else:
                    nc.vector.tensor_scalar(out=yb[:], in0=xts[s][:],
                                            scalar1=ss[:, s:s + 1],
                                            scalar2=ss[:, C + s:C + s + 1],
                                            op0=ALU.mult, op1=ALU.add)
                eng = nc.sync if s % 2 == 0 else nc.gpsimd
                eng.dma_start(out=y_slice(s), in_=yb[:])
    nc.finalize()
    return nc


def _get_nc(cap):
    key = ("nc", cap)
    if key not in _CACHE:
        _CACHE[key] = _build(cap)
    return _CACHE[key]


def kernel(x, labels, gamma, beta):
    import ml_dtypes
    from concourse.bass_utils import run_bass_kernel_spmd

    BF = ml_dtypes.bfloat16
    x = np.asarray(x, dtype=np.float32)
    lab = np.asarray(labels).astype(np.int64).ravel()
    gamma = np.asarray(gamma, dtype=np.float32)
    beta = np.asarray(beta, dtype=np.float32)

    counts = np.bincount(lab, minlength=C).astype(np.int64)
    base, rem = counts // NJ, counts % NJ
    ncj = base[None, :] + (np.arange(NJ)[:, None] < rem[None, :])  # [NJ, C]
    cap = int(-(-int(ncj.max()) // ALIGN) * ALIGN)
    cols = C * cap

    order = np.argsort(lab, kind="stable")
    starts = np.zeros(C + 1, np.int64)
    starts[1:] = np.cumsum(counts)
    # col_idx[j, c*cap + t] = original row index (N -> zero/garbage row)
    col_idx = np.full((NJ, cols), N, dtype=np.int64)
    for c in range(C):
        off = starts[c]
        for j in range(NJ):
            m = int(ncj[j, c])
            col_idx[j, c * cap:c * cap + m] = order[off:off + m]
            off += m

    xb = np.concatenate([x.astype(BF), np.zeros((1, F), BF)], axis=0)
    g = xb[col_idx.reshape(-1)]                    # [NJ*cols, F] bf16
    g = g.reshape(NJ, cols, F).transpose(0, 2, 1)  # [NJ, F, cols]

    invn = (1.0 / np.maximum(counts, 1)).astype(np.float32)
    gT, bT = gamma.T, beta.T                       # [F, C]
    selm = (np.arange(P)[:, None] % FPC == np.arange(FPC)[None, :])
    selm = np.ascontiguousarray(selm.astype(np.float32))

    nc = _get_nc(cap)
    in_maps = []
    for k in range(N_CORES):
        f0 = k * FPC
        xk = np.ascontiguousarray(g[:, f0:f0 + FPC, :]).reshape(P, cols)
        gbk = np.concatenate(
            [gT[f0:f0 + FPC], bT[f0:f0 + FPC],
             np.broadcast_to(invn[None, :], (FPC, C))], axis=1,
        ).astype(np.float32)
        in_maps.append({
            "xt": xk,
            "gbt": np.ascontiguousarray(gbk),
            "selb": selm.astype(BF),
            "self32": selm,
            "rep32": np.ascontiguousarray(selm.T),
        })
    res = run_bass_kernel_spmd(nc, in_maps, core_ids=list(range(N_CORES)),
                               **_CACHE.get("run_kwargs", {}))
    _CACHE["last_results"] = res

    ys = np.empty((N + 1, F), dtype=np.float32)    # row N absorbs padding
    for k in range(N_CORES):
        f0 = k * FPC
        yk = np.asarray(res.results[k]["y"]).reshape(NJ, FPC, cols)
        yk = yk.transpose(0, 2, 1).astype(np.float32)  # [NJ, cols, FPC]
        for j in range(NJ):
            ys[col_idx[j], f0:f0 + FPC] = yk[j]
    return np.ascontiguousarray(ys[:N])
